# revision 29
# baseline (speedup 1.0000x reference)
"""Bahdanau attention kernel for Trainium2, data-parallel over batch on 8 cores.

Problem (per full input):
  query [32,1,256], keys [32,4096,256], Wa_w/Ua_w [256,256], Wa_b/Ua_b [256],
  Va_w [1,256], Va_b [1]  ->  context [32,1,256]
  context = softmax(Va . tanh(q@Wa^T + Wa_b + k@Ua^T + Ua_b), axis=s) @ keys
  (Va_b is a constant shift over s: softmax-invariant, dropped.)

Per-core plan (4 batch elements, 8 s-panels of 512 each, batch-outer):
  A-phase per (b, panel):
    - keys panel DMA-cast f32->float32r into a unique SBUF buffer
      [s=128p, 4, 257] with a ones column at 256 (the context matmul then
      produces the softmax normalizer Z for free).
    - cast to bf16 (DVE), PE-transpose 128x128 blocks -> keysT [h,s] (bf16)
    - k_projT[o,s] = Ua^T blocks (bf16 stationary) x keysT -> PSUM f32
    - tanh with per-partition bias cb[o,b] = q_proj + Wa_b + Ua_b (ACT) -> bf16
    - scoresT[s,1] per 128-block: lhsT = actT block, rhs = Va column (bf16)
      (scores are bounded, |scores| <~ 8, so exp needs no max subtraction)
    - exp (ACT) -> eT[b] [s=128p, 32] in float32r
  B-phase per b (after all panels): one contiguous PSUM accumulation group:
    context[b] += eT-col^T @ keys-natural [128,257] (float32r), all 32
    s-blocks; col 256 accumulates Z.  Junk 1x1 transposes prime the PE's
    view of each panel's DMA semaphore so every fused f32r matmul needs
    only one sync wait (walrus limit).
  Finalize: context * (1/Z) -> out.
"""

import sys

sys.path.insert(0, "/opt/trn_rl_repo")

import numpy as np

import concourse.bass as bass
import concourse.mybir as mybir
import concourse.tile as tile
from concourse import bacc
from concourse.bass_utils import run_bass_kernel_spmd
from concourse.tile_rust import add_dep_helper

F32 = mybir.dt.float32
F32R = mybir.dt.float32r
BF16 = mybir.dt.bfloat16
AF = mybir.ActivationFunctionType

N_CORES = 8
B = 32
S = 4096
H = 256
P = 128
PANEL = 512  # s elements per panel
BL = B // N_CORES  # batch per core = 4

CTX_F32R = True  # context matmul in float32r (else bf16)

# consts blob column layout (host-prelayouted, partition dim = 128):
#   [0:128]        identity
#   [128:640]      UaT   [kh, o]   uat[p, kh*256+o] = Ua_w[o, kh*128+p]
#   [640:1152]     WaT   [kh, o]
#   [1152:1160]    qT    [kh, b]   qt[p, kh*4+b] = query[b, kh*128+p]
#   [1160:1162]    Wa_b  [oo]      wab[p, oo] = Wa_b[oo*128+p]
#   [1162:1164]    Ua_b  [oo]
#   [1164:1166]    Va    [oo]      va[p, oo] = Va_w[0, oo*128+p]
C_ID = 0
C_UAT = 128
C_WAT = 640
C_QT = 1152
C_WAB = 1160
C_UAB = 1162
C_VA = 1164
# bf16 sections, bit-packed two-per-f32-word (read on device via bitcast)
C_IDBF = 1166   # identity bf16: 64 f32 words
C_UATBF = 1230  # UaT bf16: 256 f32 words
C_VABF = 1486   # Va bf16: 1 f32 word
CONSTS_COLS = 1487

# test harness hooks: set TRACE=True before calling kernel() to capture a
# neuron-profile trace; the BassKernelResults lands in LAST_RESULT.
TRACE = False
LAST_RESULT = None


def build_core_program(s_total=S, ctx_f32r=CTX_F32R):
    """Build the per-core Bass program. s_total lets tests shrink S."""
    n_panels = s_total // PANEL
    nc = bacc.Bacc(
        "TRN2",
        target_bir_lowering=False,
        debug=False,
        enable_asserts=True,
        num_devices=N_CORES,
    )
    keys_d = nc.declare_dram_parameter("keys", [BL, s_total, H + 2], F32, isOutput=False)
    consts_d = nc.declare_dram_parameter("consts", [P, CONSTS_COLS], F32, isOutput=False)
    out_d = nc.declare_dram_parameter("out", [BL, H], F32, isOutput=True)

    with tile.TileContext(nc) as tc:
        _emit(tc, nc, keys_d, consts_d, out_d, n_panels, ctx_f32r)
    nc.compile()
    return nc


def _emit(tc, nc, keys_d, consts_d, out_d, n_panels, ctx_f32r):
    from contextlib import ExitStack

    ctx = ExitStack()
    with ctx:
        singles = ctx.enter_context(tc.tile_pool(name="singles", bufs=1))

        # ---------------- precompute ---------------------------------------
        # One HWDGE load of the host-prelayouted consts blob; weight
        # transposes were done on the host (pure layout).
        blob = singles.tile([P, CONSTS_COLS], F32)
        nc.sync.dma_start(out=blob, in_=consts_d[:, :])

        # bf16 constants come bit-packed in the blob: no conversion ops
        ident_bf = blob[:, C_IDBF:C_IDBF + P // 2].bitcast(BF16)
        uat_bf = blob[:, C_UATBF:C_UATBF + H].bitcast(BF16).rearrange(
            "p (kh o) -> p kh o", kh=2
        )
        va_bf = blob[:, C_VABF:C_VABF + 1].bitcast(BF16)
        ident_r = singles.tile([BL, BL], F32R)
        nc.vector.tensor_copy(ident_r, blob[0:BL, C_ID:C_ID + BL])

        cbt = singles.tile([P, 2, BL], F32)
        wsum = singles.tile([P, 2], F32)

        with tc.tile_pool(name="pre_ps", bufs=1, space="PSUM") as pre_ps:
            # q_proj[o, b] = sum_h Wa_w[o,h] q[b,h]  (fp32 matmuls, N=BL)
            p_qp = pre_ps.tile([P, 2, BL], F32)
            for oo in range(2):
                for kh in range(2):
                    nc.tensor.matmul(
                        p_qp[:, oo, :],
                        blob[:, C_WAT + kh * H + oo * P:C_WAT + kh * H + (oo + 1) * P],
                        blob[:, C_QT + kh * BL:C_QT + (kh + 1) * BL],
                        start=(kh == 0),
                        stop=(kh == 1),
                    )
            # cb[o, b] = q_proj[o, b] + Wa_b[o] + Ua_b[o]
            nc.vector.tensor_add(
                wsum, blob[:, C_WAB:C_WAB + 2], blob[:, C_UAB:C_UAB + 2]
            )
            for oo in range(2):
                nc.vector.tensor_scalar_add(
                    cbt[:, oo, :], p_qp[:, oo, :], wsum[:, oo:oo + 1]
                )

        # ---------------- persistent main-loop tiles ----------------------
        JS = PANEL // P  # 4 s-subblocks per panel
        KW = H + 2       # 258: keys + ones column + even-N pad
        n_iters = BL * n_panels

        if ctx_f32r:
            # unique keys buffer per (b, panel): no WAR anywhere
            knat_bufs = [
                singles.tile([P, JS, KW], F32R, tag=f"knat{k}", name=f"knat{k}")
                for k in range(n_iters)
            ]
            kbf_pool = ctx.enter_context(tc.tile_pool(name="kbf", bufs=6))
            kbf_bufs = None
        else:
            # single bf16 DMA-cast load per panel, unique buffers
            kbf_bufs = [
                singles.tile([P, JS, KW], BF16, tag=f"kbf{k}", name=f"kbf{k}")
                for k in range(n_iters)
            ]

        et = [
            singles.tile([P, JS * n_panels], F32R if ctx_f32r else BF16,
                         tag=f"et{b}", name=f"et{b}")
            for b in range(BL)
        ]

        kT_pool = ctx.enter_context(tc.tile_pool(name="kT", bufs=6))
        act_pool = ctx.enter_context(tc.tile_pool(name="act", bufs=6))

        # PSUM budget (8 banks): tr 2 + kp 3 + st 1 + ctx 2 = 8
        ps_tr = ctx.enter_context(tc.tile_pool(name="ps_tr", bufs=2, space="PSUM"))
        ps_kp = ctx.enter_context(tc.tile_pool(name="ps_kp", bufs=2, space="PSUM"))
        ps_st = ctx.enter_context(tc.tile_pool(name="ps_st", bufs=1, space="PSUM"))
        ps_ctx = ctx.enter_context(tc.tile_pool(name="ps_ctx", bufs=2, space="PSUM"))
        ps_junk = ctx.enter_context(tc.tile_pool(name="ps_junk", bufs=1, space="PSUM"))

        # rotating per-batch context accumulator at partition 0 of a bank:
        # cols 0..255 ctx, 256 = Z, 257 = even-N pad, 258..261 = junk target
        CTXW = KW + BL
        fin_pool = ctx.enter_context(tc.tile_pool(name="fin", bufs=2))

        pctx_junk = ps_junk.tile([BL, BL], F32R)

        # ---------------- main loop (batch-outer) ---------------------------
        for b in range(BL):
            panel_tiles = {}
            junks = {}
            for p_i in range(n_panels):
                k = b * n_panels + p_i
                if ctx_f32r:
                    knat = knat_bufs[k]
                    kbf = kbf_pool.tile([P, JS, KW], BF16, tag="kbf")
                    panel_tiles[p_i] = (knat, kbf)
                    # keys panel (with ones column): f32 -> f32r rounding
                    # cast during the SWDGE load
                    nc.gpsimd.dma_start(
                        out=knat,
                        in_=keys_d[b, p_i * PANEL:(p_i + 1) * PANEL, :].rearrange(
                            "(j pp) w -> pp j w", pp=P
                        ),
                    )
                    # bf16 copy for the transposes
                    nc.vector.tensor_copy(kbf, knat.bitcast(F32))
                else:
                    # all-bf16: cast during the load, no f32 copy on chip
                    knat = None
                    kbf = kbf_bufs[k]
                    panel_tiles[p_i] = (knat, kbf)
                    nc.gpsimd.dma_start(
                        out=kbf,
                        in_=keys_d[b, p_i * PANEL:(p_i + 1) * PANEL, :].rearrange(
                            "(j pp) w -> pp j w", pp=P
                        ),
                    )

                # keysT blocks via PE transpose (bf16)
                ptr = ps_tr.tile([P, 2, PANEL], BF16)
                for j in range(JS):
                    for hh in range(2):
                        nc.tensor.transpose(
                            ptr[:, hh, j * P:(j + 1) * P],
                            kbf[:, j, hh * P:(hh + 1) * P],
                            ident_bf,
                        )
                keysT = kT_pool.tile([P, 2, PANEL], BF16)
                nc.vector.tensor_copy(keysT, ptr)

                # k_projT [o, s]:  2 o-halves x 2 K-halves
                pkps = []
                for oh in range(2):
                    pkp = ps_kp.tile([P, PANEL], F32, tag="pkp")
                    pkps.append(pkp)
                    for kh in range(2):
                        nc.tensor.matmul(
                            pkp,
                            uat_bf[:, kh, oh * P:(oh + 1) * P],
                            keysT[:, kh, :],
                            start=(kh == 0),
                            stop=(kh == 1),
                        )

                # actT = tanh(k_projT + cb[o, b])  -> bf16
                actT = act_pool.tile([P, 2, PANEL], BF16)
                for oh in range(2):
                    nc.scalar.activation(
                        actT[:, oh, :],
                        pkps[oh],
                        AF.Tanh,
                        bias=cbt[:, oh, b:b + 1],
                        scale=1.0,
                    )

                # scoresT [s-block, 1] per j; accumulate over o-halves
                pst = ps_st.tile([P, JS], F32)
                for j in range(JS):
                    for oh in range(2):
                        nc.tensor.matmul(
                            pst[:, j:j + 1],
                            actT[:, oh, j * P:(j + 1) * P],
                            va_bf[:, oh:oh + 1],
                            start=(oh == 0),
                            stop=(oh == 1),
                        )

                # eT = exp(scoresT): ACT rounds to f32r (or bf16)
                nc.scalar.activation(
                    et[b][:, p_i * JS:(p_i + 1) * JS], pst[:, 0:JS], AF.Exp
                )
                if ctx_f32r:
                    # prime PE's view of this panel's DMA semaphore (fused
                    # f32r matmuls can carry only one sync wait)
                    junks[p_i] = nc.tensor.transpose(
                        pctx_junk, knat[0:BL, 0, 0:BL], ident_r
                    )

            # B-phase: one contiguous accumulation group for batch b
            pctx = ps_ctx.tile([P, CTXW], F32, tag="pctx")
            n_mm = n_panels * JS
            mi = 0
            if ctx_f32r:
                prev = None
                for p_i in range(n_panels):
                    knat, _ = panel_tiles[p_i]
                    for j in range(JS):
                        mm = nc.tensor.matmul(
                            pctx[0:1, 0:KW],
                            et[b][:, p_i * JS + j:p_i * JS + j + 1],
                            knat[:, j, :],
                            start=(mi == 0),
                            stop=(mi == n_mm - 1),
                            skip_group_check=True,
                        )
                        add_dep_helper(mm.ins, junks[p_i].ins, sync=False,
                                       reason="mm after junk")
                        if prev is not None:
                            add_dep_helper(mm.ins, prev.ins, sync=False,
                                           reason="ctx chain")
                        prev = mm
                        mi += 1
            else:
                for p_i in range(n_panels):
                    _, kbf = panel_tiles[p_i]
                    for j in range(JS):
                        nc.tensor.matmul(
                            pctx[0:1, 0:KW],
                            et[b][:, p_i * JS + j:p_i * JS + j + 1],
                            kbf[:, j, 0:KW],
                            start=(mi == 0),
                            stop=(mi == n_mm - 1),
                            skip_group_check=True,
                        )
                        mi += 1

            # finalize batch b: scale by 1/Z, store (frees the ctx bank)
            ctx_sb = fin_pool.tile([1, KW], F32, tag="ctx_sb")
            rcp = fin_pool.tile([1, 1], F32, tag="rcp")
            ctx_out = fin_pool.tile([1, H], F32, tag="ctx_out")
            nc.vector.tensor_copy(ctx_sb, pctx[0:1, 0:KW])
            nc.vector.reciprocal(rcp, ctx_sb[0:1, H:H + 1])
            nc.vector.tensor_scalar_mul(ctx_out, ctx_sb[0:1, 0:H], rcp)
            nc.sync.dma_start(out=out_d[b:b + 1, :], in_=ctx_out)


def make_consts(query_c, wa_w, wa_b, ua_w, ua_b, va_w):
    """Host-prelayouted constants blob for one core (pure layout, no math)."""
    blob = np.zeros((P, CONSTS_COLS), np.float32)
    blob[:, C_ID:C_ID + P] = np.eye(P, dtype=np.float32)
    # uat[p, kh*256+o] = Ua_w[o, kh*128+p]
    uat = ua_w.T.reshape(2, P, H).transpose(1, 0, 2)      # [p, kh, o]
    blob[:, C_UAT:C_UAT + 2 * H] = uat.reshape(P, 2 * H)
    wat = wa_w.T.reshape(2, P, H).transpose(1, 0, 2)
    blob[:, C_WAT:C_WAT + 2 * H] = wat.reshape(P, 2 * H)
    # qt[p, kh*4+b] = query_c[b, kh*128+p]
    qt = query_c.T.reshape(2, P, BL).transpose(1, 0, 2)   # [p, kh, b]
    blob[:, C_QT:C_QT + 2 * BL] = qt.reshape(P, 2 * BL)
    blob[:, C_WAB:C_WAB + 2] = wa_b.reshape(2, P).T
    blob[:, C_UAB:C_UAB + 2] = ua_b.reshape(2, P).T
    blob[:, C_VA:C_VA + 2] = va_w[0].reshape(2, P).T

    import ml_dtypes

    def pack_bf16(x):
        b = np.ascontiguousarray(x.astype(ml_dtypes.bfloat16))
        return b.view(np.uint8).reshape(x.shape[0], -1).view(np.float32)

    blob[:, C_IDBF:C_IDBF + P // 2] = pack_bf16(np.eye(P, dtype=np.float32))
    blob[:, C_UATBF:C_UATBF + H] = pack_bf16(uat.reshape(P, 2 * H))
    blob[:, C_VABF:C_VABF + 1] = pack_bf16(va_w[0].reshape(2, P).T)
    return blob


_CACHE = {}


def _get_program():
    key = (S, CTX_F32R)
    if key not in _CACHE:
        _CACHE[key] = build_core_program(ctx_f32r=CTX_F32R)
    return _CACHE[key]


def kernel(**inputs):
    query = np.asarray(inputs["query"], dtype=np.float32)   # [32, 1, 256]
    keys = np.asarray(inputs["keys"], dtype=np.float32)     # [32, 4096, 256]
    wa_w = np.asarray(inputs["Wa_w"], dtype=np.float32)
    wa_b = np.asarray(inputs["Wa_b"], dtype=np.float32)
    ua_w = np.asarray(inputs["Ua_w"], dtype=np.float32)
    ua_b = np.asarray(inputs["Ua_b"], dtype=np.float32)
    va_w = np.asarray(inputs["Va_w"], dtype=np.float32)

    nc = _get_program()
    in_maps = []
    for c in range(N_CORES):
        sl = slice(c * BL, (c + 1) * BL)
        keys_ext = np.concatenate(
            [keys[sl], np.ones((BL, keys.shape[1], 1), np.float32),
             np.zeros((BL, keys.shape[1], 1), np.float32)], axis=2
        )
        in_maps.append({
            "keys": np.ascontiguousarray(keys_ext),
            "consts": make_consts(query[sl, 0, :], wa_w, wa_b, ua_w, ua_b, va_w),
        })
    global LAST_RESULT
    res = run_bass_kernel_spmd(nc, in_maps, list(range(N_CORES)), trace=TRACE)
    LAST_RESULT = res
    out = np.concatenate([res.results[c]["out"] for c in range(N_CORES)], axis=0)
    return out.reshape(B, 1, H)


if __name__ == "__main__":
    rng = np.random.default_rng(0)
    inputs = {
        "query": rng.standard_normal((B, 1, H), dtype=np.float32),
        "keys": rng.standard_normal((B, S, H), dtype=np.float32),
        "Wa_w": rng.uniform(-1 / 16, 1 / 16, (H, H)).astype(np.float32),
        "Wa_b": rng.uniform(-1 / 16, 1 / 16, H).astype(np.float32),
        "Ua_w": rng.uniform(-1 / 16, 1 / 16, (H, H)).astype(np.float32),
        "Ua_b": rng.uniform(-1 / 16, 1 / 16, H).astype(np.float32),
        "Va_w": rng.uniform(-1 / 16, 1 / 16, (1, H)).astype(np.float32),
        "Va_b": rng.uniform(-1 / 16, 1 / 16, 1).astype(np.float32),
    }
    out = kernel(**inputs)
    print("out", out.shape, out.dtype, float(np.abs(out).max()))


# revision 31
# speedup vs baseline: 1.0112x; 1.0112x over previous
"""Bahdanau attention kernel for Trainium2, data-parallel over batch on 8 cores.

Problem (per full input):
  query [32,1,256], keys [32,4096,256], Wa_w/Ua_w [256,256], Wa_b/Ua_b [256],
  Va_w [1,256], Va_b [1]  ->  context [32,1,256]
  context = softmax(Va . tanh(q@Wa^T + Wa_b + k@Ua^T + Ua_b), axis=s) @ keys
  (Va_b is a constant shift over s: softmax-invariant, dropped.)

Per-core plan (4 batch elements, 8 s-panels of 512 each, batch-outer):
  A-phase per (b, panel):
    - keys panel DMA-cast f32->float32r into a unique SBUF buffer
      [s=128p, 4, 257] with a ones column at 256 (the context matmul then
      produces the softmax normalizer Z for free).
    - cast to bf16 (DVE), PE-transpose 128x128 blocks -> keysT [h,s] (bf16)
    - k_projT[o,s] = Ua^T blocks (bf16 stationary) x keysT -> PSUM f32
    - tanh with per-partition bias cb[o,b] = q_proj + Wa_b + Ua_b (ACT) -> bf16
    - scoresT[s,1] per 128-block: lhsT = actT block, rhs = Va column (bf16)
      (scores are bounded, |scores| <~ 8, so exp needs no max subtraction)
    - exp (ACT) -> eT[b] [s=128p, 32] in float32r
  B-phase per b (after all panels): one contiguous PSUM accumulation group:
    context[b] += eT-col^T @ keys-natural [128,257] (float32r), all 32
    s-blocks; col 256 accumulates Z.  Junk 1x1 transposes prime the PE's
    view of each panel's DMA semaphore so every fused f32r matmul needs
    only one sync wait (walrus limit).
  Finalize: context * (1/Z) -> out.
"""

import sys

sys.path.insert(0, "/opt/trn_rl_repo")

import numpy as np

import concourse.bass as bass
import concourse.mybir as mybir
import concourse.tile as tile
from concourse import bacc
from concourse.bass_utils import run_bass_kernel_spmd
from concourse.tile_rust import add_dep_helper

F32 = mybir.dt.float32
F32R = mybir.dt.float32r
BF16 = mybir.dt.bfloat16
AF = mybir.ActivationFunctionType

N_CORES = 8
B = 32
S = 4096
H = 256
P = 128
PANEL = 512  # s elements per panel
BL = B // N_CORES  # batch per core = 4

CTX_F32R = True  # context matmul in float32r (else bf16)

# consts blob column layout (host-prelayouted, partition dim = 128):
#   [0:128]        identity
#   [128:640]      UaT   [kh, o]   uat[p, kh*256+o] = Ua_w[o, kh*128+p]
#   [640:1152]     WaT   [kh, o]
#   [1152:1160]    qT    [kh, b]   qt[p, kh*4+b] = query[b, kh*128+p]
#   [1160:1162]    Wa_b  [oo]      wab[p, oo] = Wa_b[oo*128+p]
#   [1162:1164]    Ua_b  [oo]
#   [1164:1166]    Va    [oo]      va[p, oo] = Va_w[0, oo*128+p]
C_ID = 0
C_UAT = 128
C_WAT = 640
C_QT = 1152
C_WAB = 1160
C_UAB = 1162
C_VA = 1164
# bf16 sections, bit-packed two-per-f32-word (read on device via bitcast)
C_IDBF = 1166   # identity bf16: 64 f32 words
C_UATBF = 1230  # UaT bf16: 256 f32 words
C_VABF = 1486   # Va bf16: 1 f32 word
CONSTS_COLS = 1487

# test harness hooks: set TRACE=True before calling kernel() to capture a
# neuron-profile trace; the BassKernelResults lands in LAST_RESULT.
TRACE = False
LAST_RESULT = None


def build_core_program(s_total=S, ctx_f32r=CTX_F32R):
    """Build the per-core Bass program. s_total lets tests shrink S."""
    n_panels = s_total // PANEL
    nc = bacc.Bacc(
        "TRN2",
        target_bir_lowering=False,
        debug=False,
        enable_asserts=True,
        num_devices=N_CORES,
    )
    keys_d = nc.declare_dram_parameter("keys", [BL, s_total, H + 2], F32, isOutput=False)
    consts_d = nc.declare_dram_parameter("consts", [P, CONSTS_COLS], F32, isOutput=False)
    out_d = nc.declare_dram_parameter("out", [BL, H], F32, isOutput=True)

    with tile.TileContext(nc) as tc:
        _emit(tc, nc, keys_d, consts_d, out_d, n_panels, ctx_f32r)
    nc.compile()
    return nc


def _emit(tc, nc, keys_d, consts_d, out_d, n_panels, ctx_f32r):
    from contextlib import ExitStack

    ctx = ExitStack()
    with ctx:
        singles = ctx.enter_context(tc.tile_pool(name="singles", bufs=1))

        # ---------------- precompute ---------------------------------------
        # One HWDGE load of the host-prelayouted consts blob; weight
        # transposes were done on the host (pure layout).
        blob = singles.tile([P, CONSTS_COLS], F32)
        nc.sync.dma_start(out=blob, in_=consts_d[:, :])

        # bf16 constants come bit-packed in the blob: no conversion ops
        ident_bf = blob[:, C_IDBF:C_IDBF + P // 2].bitcast(BF16)
        uat_bf = blob[:, C_UATBF:C_UATBF + H].bitcast(BF16).rearrange(
            "p (kh o) -> p kh o", kh=2
        )
        va_bf = blob[:, C_VABF:C_VABF + 1].bitcast(BF16)
        ident_r = singles.tile([BL, BL], F32R)
        nc.vector.tensor_copy(ident_r, blob[0:BL, C_ID:C_ID + BL])

        cbt = singles.tile([P, 2, BL], F32)
        wsum = singles.tile([P, 2], F32)

        with tc.tile_pool(name="pre_ps", bufs=1, space="PSUM") as pre_ps:
            # q_proj[o, b] = sum_h Wa_w[o,h] q[b,h]  (fp32 matmuls, N=BL)
            p_qp = pre_ps.tile([P, 2, BL], F32)
            for oo in range(2):
                for kh in range(2):
                    nc.tensor.matmul(
                        p_qp[:, oo, :],
                        blob[:, C_WAT + kh * H + oo * P:C_WAT + kh * H + (oo + 1) * P],
                        blob[:, C_QT + kh * BL:C_QT + (kh + 1) * BL],
                        start=(kh == 0),
                        stop=(kh == 1),
                    )
            # cb[o, b] = q_proj[o, b] + Wa_b[o] + Ua_b[o]
            nc.vector.tensor_add(
                wsum, blob[:, C_WAB:C_WAB + 2], blob[:, C_UAB:C_UAB + 2]
            )
            for oo in range(2):
                nc.vector.tensor_scalar_add(
                    cbt[:, oo, :], p_qp[:, oo, :], wsum[:, oo:oo + 1]
                )

        # ---------------- persistent main-loop tiles ----------------------
        JS = PANEL // P  # 4 s-subblocks per panel
        KW = H + 2       # 258: keys + ones column + even-N pad
        n_iters = BL * n_panels

        if ctx_f32r:
            # unique keys buffer per (b, panel): no WAR anywhere
            knat_bufs = [
                singles.tile([P, JS, KW], F32R, tag=f"knat{k}", name=f"knat{k}")
                for k in range(n_iters)
            ]
            kbf_pool = ctx.enter_context(tc.tile_pool(name="kbf", bufs=6))
            kbf_bufs = None
        else:
            # single bf16 DMA-cast load per panel, unique buffers
            kbf_bufs = [
                singles.tile([P, JS, KW], BF16, tag=f"kbf{k}", name=f"kbf{k}")
                for k in range(n_iters)
            ]

        et = [
            singles.tile([P, JS * n_panels], F32R if ctx_f32r else BF16,
                         tag=f"et{b}", name=f"et{b}")
            for b in range(BL)
        ]

        kT_pool = ctx.enter_context(tc.tile_pool(name="kT", bufs=6))
        act_pool = ctx.enter_context(tc.tile_pool(name="act", bufs=6))

        # PSUM budget (8 banks): tr 2 + kp 3 + st 1 + ctx 2 = 8
        ps_tr = ctx.enter_context(tc.tile_pool(name="ps_tr", bufs=2, space="PSUM"))
        ps_kp = ctx.enter_context(tc.tile_pool(name="ps_kp", bufs=2, space="PSUM"))
        ps_st = ctx.enter_context(tc.tile_pool(name="ps_st", bufs=1, space="PSUM"))
        ps_ctx = ctx.enter_context(tc.tile_pool(name="ps_ctx", bufs=2, space="PSUM"))
        ps_junk = ctx.enter_context(tc.tile_pool(name="ps_junk", bufs=1, space="PSUM"))

        # rotating per-batch context accumulator at partition 0 of a bank:
        # cols 0..255 ctx, 256 = Z, 257 = even-N pad, 258..261 = junk target
        CTXW = KW + BL
        fin_pool = ctx.enter_context(tc.tile_pool(name="fin", bufs=2))

        pctx_junk = ps_junk.tile([BL, BL], F32R)

        # ---------------- main loop (batch-outer) ---------------------------
        for b in range(BL):
            panel_tiles = {}
            junks = {}
            for p_i in range(n_panels):
                k = b * n_panels + p_i
                if ctx_f32r:
                    knat = knat_bufs[k]
                    kbf = kbf_pool.tile([P, JS, KW], BF16, tag="kbf")
                    panel_tiles[p_i] = (knat, kbf)
                    # keys panel (with ones column): f32 -> f32r rounding
                    # cast during the SWDGE load
                    nc.gpsimd.dma_start(
                        out=knat,
                        in_=keys_d[b, p_i * PANEL:(p_i + 1) * PANEL, :].rearrange(
                            "(j pp) w -> pp j w", pp=P
                        ),
                    )
                    # bf16 copy for the transposes
                    nc.vector.tensor_copy(kbf, knat.bitcast(F32))
                else:
                    # all-bf16: cast during the load, no f32 copy on chip
                    knat = None
                    kbf = kbf_bufs[k]
                    panel_tiles[p_i] = (knat, kbf)
                    nc.gpsimd.dma_start(
                        out=kbf,
                        in_=keys_d[b, p_i * PANEL:(p_i + 1) * PANEL, :].rearrange(
                            "(j pp) w -> pp j w", pp=P
                        ),
                    )

                # keysT blocks via PE transpose (bf16)
                ptr = ps_tr.tile([P, 2, PANEL], BF16)
                for j in range(JS):
                    for hh in range(2):
                        nc.tensor.transpose(
                            ptr[:, hh, j * P:(j + 1) * P],
                            kbf[:, j, hh * P:(hh + 1) * P],
                            ident_bf,
                        )
                keysT = kT_pool.tile([P, 2, PANEL], BF16)
                nc.vector.tensor_copy(keysT, ptr)

                # k_projT [o, s]:  2 o-halves x 2 K-halves
                pkps = []
                for oh in range(2):
                    pkp = ps_kp.tile([P, PANEL], F32, tag="pkp")
                    pkps.append(pkp)
                    for kh in range(2):
                        nc.tensor.matmul(
                            pkp,
                            uat_bf[:, kh, oh * P:(oh + 1) * P],
                            keysT[:, kh, :],
                            start=(kh == 0),
                            stop=(kh == 1),
                        )

                # actT = tanh(k_projT + cb[o, b])  -> bf16
                actT = act_pool.tile([P, 2, PANEL], BF16)
                for oh in range(2):
                    nc.scalar.activation(
                        actT[:, oh, :],
                        pkps[oh],
                        AF.Tanh,
                        bias=cbt[:, oh, b:b + 1],
                        scale=1.0,
                    )

                # scoresT [s-block, 1] per j; accumulate over o-halves
                pst = ps_st.tile([P, JS], F32)
                for j in range(JS):
                    for oh in range(2):
                        nc.tensor.matmul(
                            pst[:, j:j + 1],
                            actT[:, oh, j * P:(j + 1) * P],
                            va_bf[:, oh:oh + 1],
                            start=(oh == 0),
                            stop=(oh == 1),
                        )

                # eT = exp(scoresT): ACT rounds to f32r (or bf16)
                nc.scalar.activation(
                    et[b][:, p_i * JS:(p_i + 1) * JS], pst[:, 0:JS], AF.Exp
                )
                if ctx_f32r:
                    # prime PE's view of this panel's DMA semaphore (fused
                    # f32r matmuls can carry only one sync wait)
                    junks[p_i] = nc.tensor.transpose(
                        pctx_junk, knat[0:BL, 0, 0:BL], ident_r
                    )

            # B-phase: one contiguous accumulation group for batch b
            pctx = ps_ctx.tile([P, CTXW], F32, tag="pctx")
            n_mm = n_panels * JS
            mi = 0
            if ctx_f32r:
                prev = None
                for p_i in range(n_panels):
                    knat, _ = panel_tiles[p_i]
                    for j in range(JS):
                        mm = nc.tensor.matmul(
                            pctx[0:1, 0:KW],
                            et[b][:, p_i * JS + j:p_i * JS + j + 1],
                            knat[:, j, :],
                            start=(mi == 0),
                            stop=(mi == n_mm - 1),
                            skip_group_check=True,
                        )
                        add_dep_helper(mm.ins, junks[p_i].ins, sync=False,
                                       reason="mm after junk")
                        if prev is not None:
                            add_dep_helper(mm.ins, prev.ins, sync=False,
                                           reason="ctx chain")
                        prev = mm
                        mi += 1
            else:
                for p_i in range(n_panels):
                    _, kbf = panel_tiles[p_i]
                    for j in range(JS):
                        nc.tensor.matmul(
                            pctx[0:1, 0:KW],
                            et[b][:, p_i * JS + j:p_i * JS + j + 1],
                            kbf[:, j, 0:KW],
                            start=(mi == 0),
                            stop=(mi == n_mm - 1),
                            skip_group_check=True,
                        )
                        mi += 1

            # finalize batch b: scale by 1/Z, store (frees the ctx bank)
            ctx_sb = fin_pool.tile([1, KW], F32, tag="ctx_sb")
            rcp = fin_pool.tile([1, 1], F32, tag="rcp")
            ctx_out = fin_pool.tile([1, H], F32, tag="ctx_out")
            nc.vector.tensor_copy(ctx_sb, pctx[0:1, 0:KW])
            nc.vector.reciprocal(rcp, ctx_sb[0:1, H:H + 1])
            nc.vector.tensor_scalar_mul(ctx_out, ctx_sb[0:1, 0:H], rcp)
            nc.sync.dma_start(out=out_d[b:b + 1, :], in_=ctx_out)


def make_consts(query_c, wa_w, wa_b, ua_w, ua_b, va_w):
    """Host-prelayouted constants blob for one core (pure layout, no math)."""
    blob = np.zeros((P, CONSTS_COLS), np.float32)
    blob[:, C_ID:C_ID + P] = np.eye(P, dtype=np.float32)
    # uat[p, kh*256+o] = Ua_w[o, kh*128+p]
    uat = ua_w.T.reshape(2, P, H).transpose(1, 0, 2)      # [p, kh, o]
    blob[:, C_UAT:C_UAT + 2 * H] = uat.reshape(P, 2 * H)
    wat = wa_w.T.reshape(2, P, H).transpose(1, 0, 2)
    blob[:, C_WAT:C_WAT + 2 * H] = wat.reshape(P, 2 * H)
    # qt[p, kh*4+b] = query_c[b, kh*128+p]
    qt = query_c.T.reshape(2, P, BL).transpose(1, 0, 2)   # [p, kh, b]
    blob[:, C_QT:C_QT + 2 * BL] = qt.reshape(P, 2 * BL)
    blob[:, C_WAB:C_WAB + 2] = wa_b.reshape(2, P).T
    blob[:, C_UAB:C_UAB + 2] = ua_b.reshape(2, P).T
    blob[:, C_VA:C_VA + 2] = va_w[0].reshape(2, P).T

    import ml_dtypes

    def pack_bf16(x):
        b = np.ascontiguousarray(x.astype(ml_dtypes.bfloat16))
        return b.view(np.uint8).reshape(x.shape[0], -1).view(np.float32)

    blob[:, C_IDBF:C_IDBF + P // 2] = pack_bf16(np.eye(P, dtype=np.float32))
    blob[:, C_UATBF:C_UATBF + H] = pack_bf16(uat.reshape(P, 2 * H))
    blob[:, C_VABF:C_VABF + 1] = pack_bf16(va_w[0].reshape(2, P).T)
    return blob


_CACHE = {}


def _get_program():
    key = (S, CTX_F32R)
    if key not in _CACHE:
        _CACHE[key] = build_core_program(ctx_f32r=CTX_F32R)
    return _CACHE[key]


def kernel(**inputs):
    query = np.asarray(inputs["query"], dtype=np.float32)   # [32, 1, 256]
    keys = np.asarray(inputs["keys"], dtype=np.float32)     # [32, 4096, 256]
    wa_w = np.asarray(inputs["Wa_w"], dtype=np.float32)
    wa_b = np.asarray(inputs["Wa_b"], dtype=np.float32)
    ua_w = np.asarray(inputs["Ua_w"], dtype=np.float32)
    ua_b = np.asarray(inputs["Ua_b"], dtype=np.float32)
    va_w = np.asarray(inputs["Va_w"], dtype=np.float32)

    nc = _get_program()
    in_maps = []
    for c in range(N_CORES):
        sl = slice(c * BL, (c + 1) * BL)
        keys_ext = np.concatenate(
            [keys[sl], np.ones((BL, keys.shape[1], 1), np.float32),
             np.zeros((BL, keys.shape[1], 1), np.float32)], axis=2
        )
        in_maps.append({
            "keys": np.ascontiguousarray(keys_ext),
            "consts": make_consts(query[sl, 0, :], wa_w, wa_b, ua_w, ua_b, va_w),
        })
    global LAST_RESULT
    res = run_bass_kernel_spmd(nc, in_maps, list(range(N_CORES)), trace=TRACE)
    LAST_RESULT = res
    out = np.concatenate([res.results[c]["out"] for c in range(N_CORES)], axis=0)
    return out.reshape(B, 1, H)


if __name__ == "__main__":
    rng = np.random.default_rng(0)
    inputs = {
        "query": rng.standard_normal((B, 1, H), dtype=np.float32),
        "keys": rng.standard_normal((B, S, H), dtype=np.float32),
        "Wa_w": rng.uniform(-1 / 16, 1 / 16, (H, H)).astype(np.float32),
        "Wa_b": rng.uniform(-1 / 16, 1 / 16, H).astype(np.float32),
        "Ua_w": rng.uniform(-1 / 16, 1 / 16, (H, H)).astype(np.float32),
        "Ua_b": rng.uniform(-1 / 16, 1 / 16, H).astype(np.float32),
        "Va_w": rng.uniform(-1 / 16, 1 / 16, (1, H)).astype(np.float32),
        "Va_b": rng.uniform(-1 / 16, 1 / 16, 1).astype(np.float32),
    }
    out = kernel(**inputs)
    print("out", out.shape, out.dtype, float(np.abs(out).max()))
